# revision 9
# baseline (speedup 1.0000x reference)
"""Trainium2 Bass kernel for nn_CRFModel (PAC-CRF mean-field, 5 steps).

Sharding: 8 cores = batch (2) x h-stripe (4). Full-res softmax/update are
pointwise per stripe; the blur-res pooled softmax V is AllGather'd within
each 4-core batch group every step; the 11x11 pixel-adaptive conv runs as 11
PSUM-accumulated banded matmuls (w-band x h-shift) on a linearized RGB
kernel:  K0 ~= G_spatial * (c0 - c1*||dr||^2/2)  (minimax linear, err<=5e-6).
Kernel 1 is position-only at blur res => exact fixed separable Gaussian
(truncated to 5 h-taps; tap 3 weight is 3e-4).

v6 (final, HW 1048340 ns vs 7972713 ns baseline):
 - value tensors on the V/conv path stay fp32: the mean-field winner
   selection chaotically amplifies value noise (host sim: bf16 pooled-Q ->
   rel err 0.53, fp16 -> 0.16, 12-bit -> 0.037 vs the 2e-2 budget).
 - the bilinear upsample path runs in float32r (1 cyc/row at N>=256 vs
   fp32's 4): msgb/tminb/xmg/tt34 are rounded to f32r by ACT Copy (the BIR
   verifier requires producers to round), uw/uhl are DMA'd as f32r.
 - softmax denominator via contiguous pairwise-tree adds (the strided
   c-innermost tensor_reduce was 16.6us).
 - the 4x w-pool is folded into the h-pool PE matmul (4 accumulated
   matmuls over strided moving slices) instead of a 10us DVE reduce.
 - compat runs as per-4-row stationary matmuls producing [w, h, c] directly;
   bilinear upsample via stationary-msg / stationary-Uh matmuls. No DRAM
   round-trips inside the step loop beyond the collective.
 - PSUM->SBUF copies run on the scalar engine (ACT Copy) to unload DVE.
"""
import numpy as np

C = 16; B = 2; H = W = 512; KS = 11; PAD = 5; NUM_STEPS = 5
UNARY_W = 0.8; PW0, PW1 = 2.0, 0.6; RGB_SCALE = 13.0
hb = H // 4; wb = W // 4                 # 128, 128
SH = 128                                 # full-res stripe rows
SB = 32                                  # blur-res stripe rows
NH = 44                                  # blur rows per core (34 out + 10)
NO = 34                                  # blur out rows (32 + 2 bilinear halo)
K1T = 5                                  # truncated h-taps for kernel 1
ZMAX = 3.0 * (1.0 / RGB_SCALE) ** 2 / 2.0
_c1 = (1.0 - np.exp(-ZMAX)) / ZMAX
_zs = -np.log(_c1)
_E = (1.0 - _c1 * _zs - np.exp(-_zs)) / 2.0
C0 = np.float32(1.0 - _E)
C1 = np.float32(_c1)

_CACHE = {}


def _host_consts():
    d = np.arange(-PAD, PAD + 1, dtype=np.float64)
    g0 = np.exp(-(d ** 2) / 800.0)
    g1 = np.exp(-8.0 * (d ** 2) / 9.0)

    def band(g):
        M = np.zeros((wb, wb), np.float32)
        for j in range(wb):
            for k in range(KS):
                i = j + k - PAD
                if 0 <= i < wb:
                    M[i, j] = np.float32(g[k])
        return M

    Gd0 = np.stack([np.float32(g0[k]) * band(g0) for k in range(KS)])
    # kernel 1 h-taps truncated to k = 3..7 (g1 at |d|>=3 is <= 3.4e-4)
    Gd1 = np.stack([np.float32(g1[k]) * band(g1) for k in range(3, 3 + K1T)])

    P4s = np.zeros((SH, SB), np.float32)
    for r in range(SH):
        P4s[r, r // 4] = 1.0 / 16.0

    def up_matrix(n_out, n_in):
        U = np.zeros((n_in, n_out), np.float32)
        s = n_in / n_out
        for r in range(n_out):
            y = (r + 0.5) * s - 0.5
            y0 = int(np.floor(y)); fr = np.float32(y - y0)
            U[min(max(y0, 0), n_in - 1), r] += np.float32(1) - fr
            U[min(max(y0 + 1, 0), n_in - 1), r] += fr
        return U

    Uw = up_matrix(W, wb)
    Uh_full = up_matrix(H, hb)
    Uh_loc = np.zeros((4, NO, SH), np.float32)
    for q in range(4):
        blk = Uh_full[:, SH * q: SH * (q + 1)]
        for i in range(NO):
            k = 32 * q - 1 + i
            if 0 <= k < hb:
                Uh_loc[q, i] = blk[k]
    return dict(Gd0=Gd0, Gd1=Gd1, P4s=P4s, Uw=np.ascontiguousarray(Uw),
                Uh_loc=Uh_loc)


def _build():
    import concourse.bass as bass
    import concourse.bacc as bacc
    import concourse.tile as tile
    from concourse import mybir
    from contextlib import ExitStack

    f32 = mybir.dt.float32
    f32r = mybir.dt.float32r
    bf16 = mybir.dt.float32  # BISECT: all f32
    AL = mybir.AluOpType
    ACTF = mybir.ActivationFunctionType
    X = mybir.AxisListType.X

    nc = bacc.Bacc("TRN2", target_bir_lowering=False, debug=False, num_devices=8)
    xs_d = nc.dram_tensor("xs", [SH, C, W], f32, kind="ExternalInput")
    rt_d = nc.dram_tensor("rt", [wb, 3, 46], bf16, kind="ExternalInput")
    rho_d = nc.dram_tensor("rho", [wb, 46], bf16, kind="ExternalInput")
    phi_d = nc.dram_tensor("phi", [wb, 46], bf16, kind="ExternalInput")
    w01_d = nc.dram_tensor("w01", [64, 128], bf16, kind="ExternalInput")
    uh_d = nc.dram_tensor("uh", [NO, SH], f32r, kind="ExternalInput")
    gd0_d = nc.dram_tensor("gd0", [KS, wb, wb], bf16, kind="ExternalInput")
    gd1_d = nc.dram_tensor("gd1", [K1T, wb, wb], bf16, kind="ExternalInput")
    p4s_d = nc.dram_tensor("p4s", [SH, SB], bf16, kind="ExternalInput")
    uw_d = nc.dram_tensor("uw", [wb, W], f32r, kind="ExternalInput")
    out_d = nc.dram_tensor("out", [SH, C, W], f32, kind="ExternalOutput")

    def bc(ap, n, at=1):
        """insert broadcast dim (step0 x n) at free position `at`."""
        dims = list(ap.ap)
        dims.insert(at, [0, n])
        return bass.AP(tensor=ap.tensor, offset=ap.offset, ap=dims)

    with tile.TileContext(nc) as tc, ExitStack() as ctx:
        sb = ctx.enter_context(tc.tile_pool(name="sb", bufs=1))
        sc = ctx.enter_context(tc.tile_pool(name="sc", bufs=1))
        dr = ctx.enter_context(tc.tile_pool(name="dr", bufs=1, space="DRAM"))

        q32 = nc.sync.partition_id() % 4 * 32

        logq = sb.tile([SH, C, W], f32)
        u08m = sb.tile([SH, C, W], f32)
        qb = sb.tile([SH, C, W], bf16)
        t8 = sb.tile([SH, 8, W], f32)
        gd0 = sb.tile([wb, KS, wb], bf16)
        nc.sync.dma_start(out=gd0[:], in_=gd0_d.ap().rearrange("k v w -> v k w"))
        gd1 = sb.tile([wb, K1T, wb], bf16)
        nc.sync.dma_start(out=gd1[:], in_=gd1_d.ap().rearrange("k v w -> v k w"))
        p4s = sb.tile([SH, SB], bf16); nc.sync.dma_start(out=p4s[:], in_=p4s_d.ap())
        uw = sb.tile([wb, W], f32r); nc.sync.dma_start(out=uw[:], in_=uw_d.ap())
        uhl = sb.tile([NO, SH], f32r); nc.sync.dma_start(out=uhl[:], in_=uh_d.ap())
        w01 = sb.tile([64, 128], bf16); nc.sync.dma_start(out=w01[:], in_=w01_d.ap())
        rT = sb.tile([wb, 3, 46], bf16); nc.sync.dma_start(out=rT[:], in_=rt_d.ap())
        rhoT = sb.tile([wb, 46], bf16); nc.sync.dma_start(out=rhoT[:], in_=rho_d.ap())
        phi0 = sb.tile([wb, 46], bf16); nc.sync.dma_start(out=phi0[:], in_=phi_d.ap())
        Rrec = sb.tile([SH, W], f32)

        vbounce = dr.tile([SB, C, wb], bf16)
        gpad = dr.tile([140, C, wb], bf16)

        def rowsum16(src):
            # tree-sum the 16 channels into t8[:, 0, :] (contiguous slabs)
            nc.vector.tensor_tensor(out=t8[:], in0=src[:, 0:8, :], in1=src[:, 8:16, :],
                                    op=AL.add)
            nc.vector.tensor_tensor(out=t8[:, 0:4, :], in0=t8[:, 0:4, :],
                                    in1=t8[:, 4:8, :], op=AL.add)
            nc.vector.tensor_tensor(out=t8[:, 0:2, :], in0=t8[:, 0:2, :],
                                    in1=t8[:, 2:4, :], op=AL.add)
            nc.vector.tensor_tensor(out=t8[:, 0, :], in0=t8[:, 0, :],
                                    in1=t8[:, 1, :], op=AL.add)

        # ---------- init ----------
        with tc.tile_pool(name="ini", bufs=1) as ini:
            zpad = ini.tile([96, wb], bf16)
            nc.vector.memset(zpad[:], 0.0)
            nc.sync.dma_start(out=gpad[:][0:6].rearrange("a b w -> (a b) w"), in_=zpad[:])
            nc.sync.dma_start(out=gpad[:][134:140].rearrange("a b w -> (a b) w"), in_=zpad[:])

            # unary = softmax(x)
            nc.sync.dma_start(out=logq[:], in_=xs_d.ap())
            nc.scalar.activation(out=logq[:], in_=logq[:], func=ACTF.Exp)
            rowsum16(logq)
            nc.vector.reciprocal(out=Rrec[:], in_=t8[:, 0, :])
            nc.vector.tensor_tensor(out=logq[:], in0=logq[:], in1=bc(Rrec[:], C), op=AL.mult)
            nc.vector.tensor_scalar(out=u08m[:], in0=logq[:], scalar1=UNARY_W,
                                    scalar2=UNARY_W, op0=AL.mult, op1=AL.subtract)
            nc.vector.tensor_scalar(out=logq[:], in0=logq[:], scalar1=1.0,
                                    scalar2=1.0, op0=AL.mult, op1=AL.subtract)

        # ---------- steps ----------
        for step in range(NUM_STEPS):
            last = step == NUM_STEPS - 1
            nc.scalar.activation(out=logq[:], in_=logq[:], func=ACTF.Exp)
            rowsum16(logq)
            nc.vector.reciprocal(out=Rrec[:], in_=t8[:, 0, :])
            nc.vector.tensor_tensor(out=qb[:], in0=logq[:], in1=bc(Rrec[:], C), op=AL.mult)
            # pool 4x4 + 1/16: h via P4s stationary, w via 4 accumulated
            # matmuls over strided moving slices
            with tc.tile_pool(name="psv", bufs=1, space="PSUM") as psv:
                vps = psv.tile([SB, C, wb], f32, tag="vps")
                for g in range(4):           # c-chunks keep moving free at 512
                    qs = qb[:, 4 * g:4 * (g + 1), :].rearrange("p c (v k) -> p c v k", k=4)
                    for k in range(4):
                        nc.tensor.matmul(vps[:, 4 * g:4 * (g + 1), :], p4s[:],
                                         qs[:, :, :, k], start=(k == 0), stop=(k == 3))
                vcp = sc.tile([SB, C, wb], bf16, tag="cpy2")
                nc.scalar.activation(out=vcp[:], in_=vps[:], func=ACTF.Copy)
                nc.sync.dma_start(out=vbounce[:], in_=vcp[:])
            nc.gpsimd.collective_compute(
                "AllGather", AL.bypass, replica_groups=[[0, 1, 2, 3], [4, 5, 6, 7]],
                ins=[vbounce[:].opt()], outs=[gpad[:][6:134].opt()])

            # load this core's 44 blur rows as [(4h x c), hh, w] for compat
            vc4 = sc.tile([64, 11, wb], bf16, tag="vc4")
            nc.sync.dma_start(
                out=vc4[:],
                in_=gpad[:][bass.ds(q32, 44), :, :].rearrange(
                    "(hh four) c w -> (four c) hh w", four=4))

            # compat into [w, h, c01] via stationary-V matmuls
            v01t = sc.tile([wb, NH, 32], bf16, tag="v01t")
            for t, (h0, h1) in enumerate(((0, 4), (4, 8), (8, 11))):
                with tc.tile_pool(name="psc", bufs=1, space="PSUM") as psc:
                    cpv = psc.tile([wb, 4 * (h1 - h0), 32], f32, tag="cpv")
                    for hh in range(h0, h1):
                        nc.tensor.matmul(cpv[:, 4 * (hh - h0):4 * (hh - h0 + 1), :],
                                         vc4[:, hh, :], w01[:], start=True, stop=True)
                    nc.scalar.activation(out=v01t[:, 4 * h0:4 * h1, :], in_=cpv[:], func=ACTF.Copy)

            # fields for the linearized RGB kernel
            flds = []
            for m in range(3):
                f = sc.tile([wb, NH, C], bf16, tag=f"fl{m}")
                nc.vector.tensor_tensor(out=f[:], in0=v01t[:, :, 0:16],
                                        in1=bc(rT[:, m, 1:45], C, at=2), op=AL.mult)
                flds.append(f)
            f4 = sc.tile([wb, NH, C], bf16, tag="fl4")
            nc.vector.tensor_tensor(out=f4[:], in0=v01t[:, :, 0:16],
                                    in1=bc(rhoT[:, 1:45], C, at=2), op=AL.mult)

            msg32 = sc.tile([wb, NO, C], f32, tag="msg32")
            msgb = sc.tile([wb, NO, C], bf16, tag="msgb")
            tmpm = sc.tile([wb, NO, 8], f32, tag="tmpm")
            for cf in range(2):          # c-halves: psum + moving free <= 512
              with tc.tile_pool(name="psb", bufs=1, space="PSUM") as psb:
                c0, c1k = 8 * cf, 16 + 8 * cf
                stiles = []
                for nm, srct, coff, gdt, nk in (
                        ("s0", v01t, c0, gd0, KS), ("s1", flds[0], c0, gd0, KS),
                        ("s2", flds[1], c0, gd0, KS), ("s3", flds[2], c0, gd0, KS),
                        ("s4", f4, c0, gd0, KS), ("sk", v01t, c1k, gd1, K1T)):
                    st = psb.tile([wb, NO, 8], f32, tag=nm)
                    dk = (KS - nk) // 2
                    for k in range(nk):
                        nc.tensor.matmul(st[:], gdt[:, k, :],
                                         srct[:, k + dk:k + dk + NO, coff:coff + 8],
                                         start=(k == 0), stop=(k == nk - 1))
                    stiles.append(st)
                s0, s1, s2, s3, s4, skt = stiles
                mh = msg32[:, :, 8 * cf:8 * (cf + 1)]
                nc.vector.tensor_tensor(out=mh, in0=s0[:], in1=bc(phi0[:, 6:6 + NO], 8, at=2),
                                        op=AL.mult)
                for m in range(3):
                    nc.vector.tensor_tensor(out=tmpm[:], in0=[s1, s2, s3][m][:],
                                            in1=bc(rT[:, m, 6:6 + NO], 8, at=2), op=AL.mult)
                    nc.vector.scalar_tensor_tensor(out=mh, in0=tmpm[:], scalar=float(C1),
                                                   in1=mh, op0=AL.mult, op1=AL.add)
                nc.vector.scalar_tensor_tensor(out=mh, in0=s4[:], scalar=float(-C1 / 2.0),
                                               in1=mh, op0=AL.mult, op1=AL.add)
                nc.vector.tensor_tensor(out=mh, in0=mh, in1=skt[:], op=AL.add)

            tmin = sc.tile([wb, NO], f32, tag="tmin")
            nc.vector.tensor_reduce(out=tmin[:], in_=msg32[:], axis=X, op=AL.min)
            tminb = sc.tile([wb, NO], f32r, tag="tminb")
            nc.scalar.activation(out=tminb[:], in_=tmin[:], func=ACTF.Copy)
            nc.vector.tensor_tensor(out=msgb[:], in0=msg32[:], in1=bc(tminb[:], C, at=2),
                                    op=AL.subtract)
            msgbr = sc.tile([wb, NO, C], f32r, tag="msgbr")
            nc.scalar.activation(out=msgbr[:], in_=msgb[:], func=ACTF.Copy)

            # bilinear upsample + update, 4 channels at a time:
            # W-up: msg slice is the stationary, Uw the moving -> [NO, W] per c
            # H-up: Uh_loc stationary, [NO, W] moving -> [SH, W] per c
            for g in range(4):
                xmg = sc.tile([NO, 4, W], f32r, tag="xmg")
                with tc.tile_pool(name="psw", bufs=1, space="PSUM") as psw:
                    xmp = psw.tile([NO, 4, W], f32, tag="xmp")
                    for i in range(4):
                        nc.tensor.matmul(xmp[:, i, :], msgbr[:, :, 4 * g + i], uw[:],
                                         start=True, stop=True)
                    nc.scalar.activation(out=xmg[:], in_=xmp[:], func=ACTF.Copy)
                with tc.tile_pool(name="psh", bufs=1, space="PSUM") as psh:
                    xph = psh.tile([SH, 4, W], f32, tag="xph")
                    for i in range(4):
                        nc.tensor.matmul(xph[:, i, :], uhl[:], xmg[:, i, :],
                                         start=True, stop=True)
                    nc.vector.scalar_tensor_tensor(
                        out=logq[:, 4 * g:4 * (g + 1), :], in0=xph[:], scalar=-1.0,
                        in1=u08m[:, 4 * g:4 * (g + 1), :], op0=AL.mult, op1=AL.add)
            if last:
                tt34 = sc.tile([NO, W], f32r, tag="tt34")
                with tc.tile_pool(name="pst", bufs=1, space="PSUM") as pst:
                    tmp_ = pst.tile([NO, W], f32, tag="twp")
                    nc.tensor.matmul(tmp_[:], tminb[:], uw[:], start=True, stop=True)
                    nc.scalar.activation(out=tt34[:], in_=tmp_[:], func=ACTF.Copy)
                    tp = pst.tile([SH, W], f32, tag="tp")
                    nc.tensor.matmul(tp[:], uhl[:], tt34[:], start=True, stop=True)
                    upt = sc.tile([SH, W], f32, tag="upt")
                    nc.vector.tensor_scalar(out=upt[:], in0=tp[:], scalar1=-1.0,
                                            scalar2=UNARY_W, op0=AL.mult, op1=AL.add)
                nc.vector.tensor_tensor(out=logq[:], in0=logq[:], in1=bc(upt[:], C),
                                        op=AL.add)

        nc.sync.dma_start(out=out_d.ap(), in_=logq[:])

    nc.compile()
    return nc


def kernel(x, image, w_compat0, w_compat1):
    import ml_dtypes
    from concourse import bass_utils
    bfloat16 = ml_dtypes.bfloat16

    if "nc" not in _CACHE:
        _CACHE["consts"] = _host_consts()
        _CACHE["nc"] = _build()
    nc = _CACHE["nc"]
    cst = _CACHE["consts"]

    x = np.ascontiguousarray(x, np.float32)
    image = np.ascontiguousarray(image, np.float32)

    w01 = np.zeros((64, 128), np.float32)
    for f in range(4):
        w01[16 * f:16 * f + 16, 32 * f:32 * f + 16] = (PW0 * w_compat0).T
        w01[16 * f:16 * f + 16, 32 * f + 16:32 * f + 32] = (PW1 * w_compat1).T
    w01 = w01.astype(np.float32)

    in_maps = []
    for cid in range(8):
        b, q = cid // 4, cid % 4
        r0 = 128 * q
        # image window: full-res rows r0-28 .. r0+156 (46 blur rows), /13
        ie = np.zeros((3, 184, W), np.float32)
        lo, hi = r0 - 28, r0 + 156
        slo, shi = max(lo, 0), min(hi, H)
        ie[:, slo - lo:shi - lo, :] = image[b, :, slo:shi, :] / np.float32(RGB_SCALE)
        ip = ie.reshape(3, 46, 4, wb, 4).mean(axis=(2, 4))      # (3, 46, wb)
        rt = np.ascontiguousarray(ip.transpose(2, 0, 1))        # (wb, 3, 46)
        rho = np.ascontiguousarray((rt ** 2).sum(axis=1))       # (wb, 46)
        phi = (C0 - C1 / 2.0 * rho).astype(np.float32)
        in_maps.append({
            "xs": np.ascontiguousarray(x[b, :, r0:r0 + 128, :].transpose(1, 0, 2)),
            "rt": rt.astype(np.float32), "rho": rho.astype(np.float32),
            "phi": phi.astype(np.float32),
            "w01": w01,
            "uh": cst["Uh_loc"][q].astype(np.float32),
            "gd0": cst["Gd0"].astype(np.float32), "gd1": cst["Gd1"].astype(np.float32),
            "p4s": cst["P4s"].astype(np.float32), "uw": cst["Uw"].astype(np.float32),
        })
    res = bass_utils.run_bass_kernel_spmd(nc, in_maps, core_ids=list(range(8)),
                                          **_CACHE.get("run_kwargs", {}))
    _CACHE["last_result"] = res
    out = np.empty((B, C, H, W), np.float32)
    for cid in range(8):
        b, q = cid // 4, cid % 4
        out[b, :, 128 * q:128 * (q + 1), :] = res.results[cid]["out"].transpose(1, 0, 2)
    return out


# revision 10
# speedup vs baseline: 1.0754x; 1.0754x over previous
"""Trainium2 Bass kernel for nn_CRFModel (PAC-CRF mean-field, 5 steps).

Sharding: 8 cores = batch (2) x h-stripe (4). Full-res softmax/update are
pointwise per stripe; the blur-res pooled softmax V is AllGather'd within
each 4-core batch group every step; the 11x11 pixel-adaptive conv runs as 11
PSUM-accumulated banded matmuls (w-band x h-shift) on a linearized RGB
kernel:  K0 ~= G_spatial * (c0 - c1*||dr||^2/2)  (minimax linear, err<=5e-6).
Kernel 1 is position-only at blur res => exact fixed separable Gaussian
(truncated to 5 h-taps; tap 3 weight is 3e-4).

v6 (final, HW 1048340 ns vs 7972713 ns baseline):
 - value tensors on the V/conv path stay fp32: the mean-field winner
   selection chaotically amplifies value noise (host sim: bf16 pooled-Q ->
   rel err 0.53, fp16 -> 0.16, 12-bit -> 0.037 vs the 2e-2 budget).
 - the bilinear upsample path runs in float32r (1 cyc/row at N>=256 vs
   fp32's 4): msgb/tminb/xmg/tt34 are rounded to f32r by ACT Copy (the BIR
   verifier requires producers to round), uw/uhl are DMA'd as f32r.
 - softmax denominator via contiguous pairwise-tree adds (the strided
   c-innermost tensor_reduce was 16.6us).
 - the 4x w-pool is folded into the h-pool PE matmul (4 accumulated
   matmuls over strided moving slices) instead of a 10us DVE reduce.
 - compat runs as per-4-row stationary matmuls producing [w, h, c] directly;
   bilinear upsample via stationary-msg / stationary-Uh matmuls. No DRAM
   round-trips inside the step loop beyond the collective.
 - PSUM->SBUF copies run on the scalar engine (ACT Copy) to unload DVE.
"""
import numpy as np

C = 16; B = 2; H = W = 512; KS = 11; PAD = 5; NUM_STEPS = 5
UNARY_W = 0.8; PW0, PW1 = 2.0, 0.6; RGB_SCALE = 13.0
hb = H // 4; wb = W // 4                 # 128, 128
SH = 128                                 # full-res stripe rows
SB = 32                                  # blur-res stripe rows
NH = 44                                  # blur rows per core (34 out + 10)
NO = 34                                  # blur out rows (32 + 2 bilinear halo)
K1T = 5                                  # truncated h-taps for kernel 1
ZMAX = 3.0 * (1.0 / RGB_SCALE) ** 2 / 2.0
_c1 = (1.0 - np.exp(-ZMAX)) / ZMAX
_zs = -np.log(_c1)
_E = (1.0 - _c1 * _zs - np.exp(-_zs)) / 2.0
C0 = np.float32(1.0 - _E)
C1 = np.float32(_c1)

_CACHE = {}


def _host_consts():
    d = np.arange(-PAD, PAD + 1, dtype=np.float64)
    g0 = np.exp(-(d ** 2) / 800.0)
    g1 = np.exp(-8.0 * (d ** 2) / 9.0)

    def band(g):
        M = np.zeros((wb, wb), np.float32)
        for j in range(wb):
            for k in range(KS):
                i = j + k - PAD
                if 0 <= i < wb:
                    M[i, j] = np.float32(g[k])
        return M

    Gd0 = np.stack([np.float32(g0[k]) * band(g0) for k in range(KS)])
    # kernel 1 h-taps truncated to k = 3..7 (g1 at |d|>=3 is <= 3.4e-4)
    Gd1 = np.stack([np.float32(g1[k]) * band(g1) for k in range(3, 3 + K1T)])

    P4s = np.zeros((SH, SB), np.float32)
    for r in range(SH):
        P4s[r, r // 4] = 1.0 / 16.0

    def up_matrix(n_out, n_in):
        U = np.zeros((n_in, n_out), np.float32)
        s = n_in / n_out
        for r in range(n_out):
            y = (r + 0.5) * s - 0.5
            y0 = int(np.floor(y)); fr = np.float32(y - y0)
            U[min(max(y0, 0), n_in - 1), r] += np.float32(1) - fr
            U[min(max(y0 + 1, 0), n_in - 1), r] += fr
        return U

    Uw = up_matrix(W, wb)
    Uh_full = up_matrix(H, hb)
    Uh_loc = np.zeros((4, NO, SH), np.float32)
    for q in range(4):
        blk = Uh_full[:, SH * q: SH * (q + 1)]
        for i in range(NO):
            k = 32 * q - 1 + i
            if 0 <= k < hb:
                Uh_loc[q, i] = blk[k]
    return dict(Gd0=Gd0, Gd1=Gd1, P4s=P4s, Uw=np.ascontiguousarray(Uw),
                Uh_loc=Uh_loc)


def _build():
    import concourse.bass as bass
    import concourse.bacc as bacc
    import concourse.tile as tile
    from concourse import mybir
    from contextlib import ExitStack

    f32 = mybir.dt.float32
    f32r = mybir.dt.float32r
    bf16 = mybir.dt.float32  # BISECT: all f32
    AL = mybir.AluOpType
    ACTF = mybir.ActivationFunctionType
    X = mybir.AxisListType.X

    nc = bacc.Bacc("TRN2", target_bir_lowering=False, debug=False, num_devices=8)
    xs_d = nc.dram_tensor("xs", [SH, C, W], f32, kind="ExternalInput")
    rt_d = nc.dram_tensor("rt", [wb, 3, 46], bf16, kind="ExternalInput")
    rho_d = nc.dram_tensor("rho", [wb, 46], bf16, kind="ExternalInput")
    phi_d = nc.dram_tensor("phi", [wb, 46], bf16, kind="ExternalInput")
    w01_d = nc.dram_tensor("w01", [64, 128], bf16, kind="ExternalInput")
    uh_d = nc.dram_tensor("uh", [NO, SH], f32r, kind="ExternalInput")
    gd0_d = nc.dram_tensor("gd0", [KS, wb, wb], f32r, kind="ExternalInput")
    gd1_d = nc.dram_tensor("gd1", [K1T, wb, wb], f32r, kind="ExternalInput")
    p4s_d = nc.dram_tensor("p4s", [SH, SB], bf16, kind="ExternalInput")
    uw_d = nc.dram_tensor("uw", [wb, W], f32r, kind="ExternalInput")
    out_d = nc.dram_tensor("out", [SH, C, W], f32, kind="ExternalOutput")

    def bc(ap, n, at=1):
        """insert broadcast dim (step0 x n) at free position `at`."""
        dims = list(ap.ap)
        dims.insert(at, [0, n])
        return bass.AP(tensor=ap.tensor, offset=ap.offset, ap=dims)

    with tile.TileContext(nc) as tc, ExitStack() as ctx:
        sb = ctx.enter_context(tc.tile_pool(name="sb", bufs=1))
        sc = ctx.enter_context(tc.tile_pool(name="sc", bufs=1))
        dr = ctx.enter_context(tc.tile_pool(name="dr", bufs=1, space="DRAM"))

        q32 = nc.sync.partition_id() % 4 * 32

        logq = sb.tile([SH, C, W], f32)
        u08m = sb.tile([SH, C, W], f32)
        t8 = sb.tile([SH, 8, W], f32)
        gd0 = sb.tile([wb, KS, wb], f32r)
        nc.sync.dma_start(out=gd0[:], in_=gd0_d.ap().rearrange("k v w -> v k w"))
        gd1 = sb.tile([wb, K1T, wb], f32r)
        nc.sync.dma_start(out=gd1[:], in_=gd1_d.ap().rearrange("k v w -> v k w"))
        p4s = sb.tile([SH, SB], bf16); nc.sync.dma_start(out=p4s[:], in_=p4s_d.ap())
        uw = sb.tile([wb, W], f32r); nc.sync.dma_start(out=uw[:], in_=uw_d.ap())
        uhl = sb.tile([NO, SH], f32r); nc.sync.dma_start(out=uhl[:], in_=uh_d.ap())
        w01 = sb.tile([64, 128], bf16); nc.sync.dma_start(out=w01[:], in_=w01_d.ap())
        rT = sb.tile([wb, 3, 46], bf16); nc.sync.dma_start(out=rT[:], in_=rt_d.ap())
        rhoT = sb.tile([wb, 46], bf16); nc.sync.dma_start(out=rhoT[:], in_=rho_d.ap())
        phi0 = sb.tile([wb, 46], bf16); nc.sync.dma_start(out=phi0[:], in_=phi_d.ap())
        Rrec = sb.tile([SH, W], f32)

        vbounce = dr.tile([SB, C, wb], bf16)
        gpad = dr.tile([140, C, wb], bf16)

        def rowsum16(src):
            # tree-sum the 16 channels into t8[:, 0, :] (contiguous slabs)
            nc.vector.tensor_tensor(out=t8[:], in0=src[:, 0:8, :], in1=src[:, 8:16, :],
                                    op=AL.add)
            nc.vector.tensor_tensor(out=t8[:, 0:4, :], in0=t8[:, 0:4, :],
                                    in1=t8[:, 4:8, :], op=AL.add)
            nc.vector.tensor_tensor(out=t8[:, 0:2, :], in0=t8[:, 0:2, :],
                                    in1=t8[:, 2:4, :], op=AL.add)
            nc.vector.tensor_tensor(out=t8[:, 0, :], in0=t8[:, 0, :],
                                    in1=t8[:, 1, :], op=AL.add)

        # ---------- init ----------
        with tc.tile_pool(name="ini", bufs=1) as ini:
            zpad = ini.tile([96, wb], bf16)
            nc.vector.memset(zpad[:], 0.0)
            nc.sync.dma_start(out=gpad[:][0:6].rearrange("a b w -> (a b) w"), in_=zpad[:])
            nc.sync.dma_start(out=gpad[:][134:140].rearrange("a b w -> (a b) w"), in_=zpad[:])

            # unary = softmax(x)
            nc.sync.dma_start(out=logq[:], in_=xs_d.ap())
            nc.scalar.activation(out=logq[:], in_=logq[:], func=ACTF.Exp)
            rowsum16(logq)
            nc.vector.reciprocal(out=Rrec[:], in_=t8[:, 0, :])
            nc.vector.tensor_tensor(out=logq[:], in0=logq[:], in1=bc(Rrec[:], C), op=AL.mult)
            nc.vector.tensor_scalar(out=u08m[:], in0=logq[:], scalar1=UNARY_W,
                                    scalar2=UNARY_W, op0=AL.mult, op1=AL.subtract)
            nc.vector.tensor_scalar(out=logq[:], in0=logq[:], scalar1=1.0,
                                    scalar2=1.0, op0=AL.mult, op1=AL.subtract)

        # ---------- steps ----------
        for step in range(NUM_STEPS):
            last = step == NUM_STEPS - 1
            nc.scalar.activation(out=logq[:], in_=logq[:], func=ACTF.Exp)
            rowsum16(logq)
            nc.vector.reciprocal(out=Rrec[:], in_=t8[:, 0, :])
            nc.vector.tensor_tensor(out=logq[:], in0=logq[:], in1=bc(Rrec[:], C), op=AL.mult)
            # pool 4x4 + 1/16: h via P4s stationary, w via 4 accumulated
            # matmuls over strided moving slices
            with tc.tile_pool(name="psv", bufs=1, space="PSUM") as psv:
                vps = psv.tile([SB, C, wb], f32, tag="vps")
                for g in range(4):           # c-chunks keep moving free at 512
                    qs = logq[:, 4 * g:4 * (g + 1), :].rearrange("p c (v k) -> p c v k", k=4)
                    for k in range(4):
                        nc.tensor.matmul(vps[:, 4 * g:4 * (g + 1), :], p4s[:],
                                         qs[:, :, :, k], start=(k == 0), stop=(k == 3))
                vcp = sc.tile([SB, C, wb], bf16, tag="cpy2")
                nc.scalar.activation(out=vcp[:], in_=vps[:], func=ACTF.Copy)
                nc.sync.dma_start(out=vbounce[:], in_=vcp[:])
            nc.gpsimd.collective_compute(
                "AllGather", AL.bypass, replica_groups=[[0, 1, 2, 3], [4, 5, 6, 7]],
                ins=[vbounce[:].opt()], outs=[gpad[:][6:134].opt()])

            # load this core's 44 blur rows as [(4h x c), hh, w] for compat
            vc4 = sc.tile([64, 11, wb], bf16, tag="vc4")
            nc.sync.dma_start(
                out=vc4[:],
                in_=gpad[:][bass.ds(q32, 44), :, :].rearrange(
                    "(hh four) c w -> (four c) hh w", four=4))

            # compat into [w, h, c01] via stationary-V matmuls
            v01t = sc.tile([wb, NH, 32], bf16, tag="v01t")
            for t, (h0, h1) in enumerate(((0, 4), (4, 8), (8, 11))):
                with tc.tile_pool(name="psc", bufs=1, space="PSUM") as psc:
                    cpv = psc.tile([wb, 4 * (h1 - h0), 32], f32, tag="cpv")
                    for hh in range(h0, h1):
                        nc.tensor.matmul(cpv[:, 4 * (hh - h0):4 * (hh - h0 + 1), :],
                                         vc4[:, hh, :], w01[:], start=True, stop=True)
                    nc.scalar.activation(out=v01t[:, 4 * h0:4 * h1, :], in_=cpv[:], func=ACTF.Copy)

            # fields for the linearized RGB kernel
            flds = []
            for m in range(3):
                f = sc.tile([wb, NH, C], bf16, tag=f"fl{m}")
                nc.vector.tensor_tensor(out=f[:], in0=v01t[:, :, 0:16],
                                        in1=bc(rT[:, m, 1:45], C, at=2), op=AL.mult)
                flds.append(f)
            f4 = sc.tile([wb, NH, C], bf16, tag="fl4")
            nc.vector.tensor_tensor(out=f4[:], in0=v01t[:, :, 0:16],
                                    in1=bc(rhoT[:, 1:45], C, at=2), op=AL.mult)

            # hi/lo f32r split: hi = f32r(x), lo = f32r(x - hi); G*hi + G*lo
            # accumulated in the same PSUM group gives fp32-quality values at
            # 1 cyc/row per pass (vs 4 cyc/row for the fp32 2-pass matmul).
            lo32 = sc.tile([wb, NH, 32], f32, tag="lo32")
            def split_hl(srct, shape, tg):
                hi = sc.tile(shape, f32r, tag=tg + "h")
                nc.scalar.activation(out=hi[:], in_=srct[:], func=ACTF.Copy)
                l32 = lo32[:, :, 0:shape[2]]
                nc.vector.tensor_tensor(out=l32, in0=srct[:], in1=hi[:],
                                        op=AL.subtract)
                lo = sc.tile(shape, f32r, tag=tg + "l")
                nc.scalar.activation(out=lo[:], in_=l32, func=ACTF.Copy)
                return hi, lo
            v01hl = split_hl(v01t, [wb, NH, 32], "v01")
            flhl = [split_hl(flds[m], [wb, NH, C], f"g{m}") for m in range(3)]
            f4hl = split_hl(f4, [wb, NH, C], "g4")

            msg32 = sc.tile([wb, NO, C], f32, tag="msg32")
            msgb = sc.tile([wb, NO, C], bf16, tag="msgb")
            tmpm = sc.tile([wb, NO, 8], f32, tag="tmpm")
            for cf in range(2):          # c-halves: psum + moving free <= 512
              with tc.tile_pool(name="psb", bufs=1, space="PSUM") as psb:
                c0, c1k = 8 * cf, 16 + 8 * cf
                stiles = []
                for nm, srhl, coff, gdt, nk in (
                        ("s0", v01hl, c0, gd0, KS), ("s1", flhl[0], c0, gd0, KS),
                        ("s2", flhl[1], c0, gd0, KS), ("s3", flhl[2], c0, gd0, KS),
                        ("s4", f4hl, c0, gd0, KS), ("sk", v01hl, c1k, gd1, K1T)):
                    st = psb.tile([wb, NO, 8], f32, tag=nm)
                    dk = (KS - nk) // 2
                    for k in range(nk):
                        for hl in range(2):
                            nc.tensor.matmul(
                                st[:], gdt[:, k, :],
                                srhl[hl][:, k + dk:k + dk + NO, coff:coff + 8],
                                start=(k == 0 and hl == 0),
                                stop=(k == nk - 1 and hl == 1))
                    stiles.append(st)
                s0, s1, s2, s3, s4, skt = stiles
                mh = msg32[:, :, 8 * cf:8 * (cf + 1)]
                nc.vector.tensor_tensor(out=mh, in0=s0[:], in1=bc(phi0[:, 6:6 + NO], 8, at=2),
                                        op=AL.mult)
                for m in range(3):
                    nc.vector.tensor_tensor(out=tmpm[:], in0=[s1, s2, s3][m][:],
                                            in1=bc(rT[:, m, 6:6 + NO], 8, at=2), op=AL.mult)
                    nc.vector.scalar_tensor_tensor(out=mh, in0=tmpm[:], scalar=float(C1),
                                                   in1=mh, op0=AL.mult, op1=AL.add)
                nc.vector.scalar_tensor_tensor(out=mh, in0=s4[:], scalar=float(-C1 / 2.0),
                                               in1=mh, op0=AL.mult, op1=AL.add)
                nc.vector.tensor_tensor(out=mh, in0=mh, in1=skt[:], op=AL.add)

            tmin = sc.tile([wb, NO], f32, tag="tmin")
            nc.vector.tensor_reduce(out=tmin[:], in_=msg32[:], axis=X, op=AL.min)
            tminb = sc.tile([wb, NO], f32r, tag="tminb")
            nc.scalar.activation(out=tminb[:], in_=tmin[:], func=ACTF.Copy)
            nc.vector.tensor_tensor(out=msgb[:], in0=msg32[:], in1=bc(tminb[:], C, at=2),
                                    op=AL.subtract)
            msgbr = sc.tile([wb, NO, C], f32r, tag="msgbr")
            nc.scalar.activation(out=msgbr[:], in_=msgb[:], func=ACTF.Copy)

            # bilinear upsample + update, 4 channels at a time:
            # W-up: msg slice is the stationary, Uw the moving -> [NO, W] per c
            # H-up: Uh_loc stationary, [NO, W] moving -> [SH, W] per c
            for g in range(4):
                xmg = sc.tile([NO, 4, W], f32r, tag="xmg")
                with tc.tile_pool(name="psw", bufs=1, space="PSUM") as psw:
                    xmp = psw.tile([NO, 4, W], f32, tag="xmp")
                    for i in range(4):
                        nc.tensor.matmul(xmp[:, i, :], msgbr[:, :, 4 * g + i], uw[:],
                                         start=True, stop=True)
                    nc.scalar.activation(out=xmg[:], in_=xmp[:], func=ACTF.Copy)
                with tc.tile_pool(name="psh", bufs=1, space="PSUM") as psh:
                    xph = psh.tile([SH, 4, W], f32, tag="xph")
                    for i in range(4):
                        nc.tensor.matmul(xph[:, i, :], uhl[:], xmg[:, i, :],
                                         start=True, stop=True)
                    nc.vector.scalar_tensor_tensor(
                        out=logq[:, 4 * g:4 * (g + 1), :], in0=xph[:], scalar=-1.0,
                        in1=u08m[:, 4 * g:4 * (g + 1), :], op0=AL.mult, op1=AL.add)
            if last:
                tt34 = sc.tile([NO, W], f32r, tag="tt34")
                with tc.tile_pool(name="pst", bufs=1, space="PSUM") as pst:
                    tmp_ = pst.tile([NO, W], f32, tag="twp")
                    nc.tensor.matmul(tmp_[:], tminb[:], uw[:], start=True, stop=True)
                    nc.scalar.activation(out=tt34[:], in_=tmp_[:], func=ACTF.Copy)
                    tp = pst.tile([SH, W], f32, tag="tp")
                    nc.tensor.matmul(tp[:], uhl[:], tt34[:], start=True, stop=True)
                    upt = sc.tile([SH, W], f32, tag="upt")
                    nc.vector.tensor_scalar(out=upt[:], in0=tp[:], scalar1=-1.0,
                                            scalar2=UNARY_W, op0=AL.mult, op1=AL.add)
                nc.vector.tensor_tensor(out=logq[:], in0=logq[:], in1=bc(upt[:], C),
                                        op=AL.add)

        nc.sync.dma_start(out=out_d.ap(), in_=logq[:])

    nc.compile()
    return nc


def kernel(x, image, w_compat0, w_compat1):
    import ml_dtypes
    from concourse import bass_utils
    bfloat16 = ml_dtypes.bfloat16

    if "nc" not in _CACHE:
        _CACHE["consts"] = _host_consts()
        _CACHE["nc"] = _build()
    nc = _CACHE["nc"]
    cst = _CACHE["consts"]

    x = np.ascontiguousarray(x, np.float32)
    image = np.ascontiguousarray(image, np.float32)

    w01 = np.zeros((64, 128), np.float32)
    for f in range(4):
        w01[16 * f:16 * f + 16, 32 * f:32 * f + 16] = (PW0 * w_compat0).T
        w01[16 * f:16 * f + 16, 32 * f + 16:32 * f + 32] = (PW1 * w_compat1).T
    w01 = w01.astype(np.float32)

    in_maps = []
    for cid in range(8):
        b, q = cid // 4, cid % 4
        r0 = 128 * q
        # image window: full-res rows r0-28 .. r0+156 (46 blur rows), /13
        ie = np.zeros((3, 184, W), np.float32)
        lo, hi = r0 - 28, r0 + 156
        slo, shi = max(lo, 0), min(hi, H)
        ie[:, slo - lo:shi - lo, :] = image[b, :, slo:shi, :] / np.float32(RGB_SCALE)
        ip = ie.reshape(3, 46, 4, wb, 4).mean(axis=(2, 4))      # (3, 46, wb)
        rt = np.ascontiguousarray(ip.transpose(2, 0, 1))        # (wb, 3, 46)
        rho = np.ascontiguousarray((rt ** 2).sum(axis=1))       # (wb, 46)
        phi = (C0 - C1 / 2.0 * rho).astype(np.float32)
        in_maps.append({
            "xs": np.ascontiguousarray(x[b, :, r0:r0 + 128, :].transpose(1, 0, 2)),
            "rt": rt.astype(np.float32), "rho": rho.astype(np.float32),
            "phi": phi.astype(np.float32),
            "w01": w01,
            "uh": cst["Uh_loc"][q].astype(np.float32),
            "gd0": cst["Gd0"].astype(np.float32), "gd1": cst["Gd1"].astype(np.float32),
            "p4s": cst["P4s"].astype(np.float32), "uw": cst["Uw"].astype(np.float32),
        })
    res = bass_utils.run_bass_kernel_spmd(nc, in_maps, core_ids=list(range(8)),
                                          **_CACHE.get("run_kwargs", {}))
    _CACHE["last_result"] = res
    out = np.empty((B, C, H, W), np.float32)
    for cid in range(8):
        b, q = cid // 4, cid % 4
        out[b, :, 128 * q:128 * (q + 1), :] = res.results[cid]["out"].transpose(1, 0, 2)
    return out
